# revision 34
# baseline (speedup 1.0000x reference)
"""TRN2 Bass kernel for nn_Attention_5720896438407 (8-core data-parallel).

Mathematical collapse: the module computes SDPA over the *head* axis with a
single KV head (KV=1), so every attention weight is exactly 1.0 and the whole
module reduces to (see kernel_baseline.py.bak for the derivation)

    T  = hidden @ kv_a_w.T + kv_a_b                    # (ntok, 512)
    s  = rsqrt(mean(T^2, -1) + eps)                    # per-token RMS scale
    V  = (s*T) @ Wv' + bv,   Wv' = (kv_b_w[128:256] * (1+kv_norm_w)).T
    Y  = V @ M.T,            M   = o_w.reshape(2048, 16, 128).sum(1)

This version additionally *folds the value path past the RMS norm*: since s
is a per-token scalar,

    V = s * (hidden @ Wf) + bv,   Wf = kv_a_w.T @ Wv'   (2048 x 128)

so T is only needed for the statistic s.  That lets the dominant matmul
(hidden @ kv_a_w.T, 2048-dim contraction, 512 outputs) run in fp8 DoubleRow
mode (2 MACs/cell/cycle): fp8 quantization errors are independent across the
512 columns, so the *mean* of T^2 — and hence s — keeps ~0.2% accuracy while
the matmul runs at 2x rate.  The accurate value path is the cheap rank-128
fold (hidden @ Wf, fp16) plus the output projection (V @ M.T, fp16).
Numerically verified: ~2e-3 rel-fro error vs the fp32 reference.

Distribution: pure data-parallel over the 8192 tokens — 1024 per core, no
collectives.  Per core, tokens stream in 8 slabs of 128 (two 512-token
super-slabs); X ships twice (fp16 for the value path, pre-cast fp8 for the
statistic path — on-chip casting was measured far too slow on every
engine).  Per slab the PE does 8 DoubleRow MMs (statistic), 8 fp16 MMs
(value, batched per half-super-slab so it unblocks as X16 slabs land) and
4 fp16 MMs (output); ACT does the Square+accum statistic and half of each
Y PSUM->SBUF copy; DVE does s=1/sqrt and the other half, both applying the
per-token s during the copy.  The two HWDGE rings (SP/ACT) carry the
inputs in PE-consumption order and each Y half ships on the ring fed by
its producing engine.  Junk matmuls bridge the DMA-paced head so the HAM
activity monitor keeps the PE clock at 2.4 GHz (cold starts at 1.2 GHz and
re-throttles after ~3.4 us of idle — the dominant source of run-to-run
variance).
"""
import sys

sys.path.insert(0, "/opt/trn_rl_repo")

import numpy as np
import ml_dtypes
import concourse.tile as tile
from concourse import bacc, mybir
from concourse.bass_utils import run_bass_kernel_spmd

F32 = mybir.dt.float32
F16 = mybir.dt.float16
F8 = mybir.dt.float8e4
DRMODE = mybir.MatmulPerfMode.DoubleRow
AF = mybir.ActivationFunctionType

HID = 2048
KV = 512
D = 128
OUT = 2048
EPS = 1e-6
WSC = 64.0                    # fp8 scale on kv_a_w (entries ~0.02)
SQ_SCALE = 1.0 / (KV * WSC * WSC)
SLAB = 128                    # tokens per slab
SS_TOK = 512                  # tokens per super-slab (value-matmul batch)
N_CORES = 8
E4 = ml_dtypes.float8_e4m3

_NC_CACHE = {}


def _build_nc(tok, with_ba):
    nss = tok // SS_TOK
    assert tok % SS_TOK == 0 and nss == 2

    nc = bacc.Bacc("TRN2", target_bir_lowering=False, debug=False,
                   num_devices=1)

    x16_d = nc.dram_tensor("x16", (nss, 128, 4, 16, SLAB), F16,
                           kind="ExternalInput").ap()
    x8p_d = nc.dram_tensor("x8p", (nss, 128, 4, 8, 2, SLAB), F8,
                           kind="ExternalInput").ap()
    w18_d = nc.dram_tensor("w18", (128, 8, 2, KV), F8,
                           kind="ExternalInput").ap()
    wf_d = nc.dram_tensor("wf", (128, 16, D), F16, kind="ExternalInput").ap()
    mt_d = nc.dram_tensor("mt", (D, OUT), F16, kind="ExternalInput").ap()
    if with_ba:
        bar8_d = nc.dram_tensor("bar8", (1, KV), F8,
                                kind="ExternalInput").ap()
        cvt_d = nc.dram_tensor("cvt", (1, D), F16, kind="ExternalInput").ap()
    y_d = nc.dram_tensor("y", (tok, OUT), F16, kind="ExternalOutput").ap()

    with tile.TileContext(nc) as tc:
        with tc.tile_pool(name="consts", bufs=1) as consts, \
             tc.tile_pool(name="xs16", bufs=2) as xs16, \
             tc.tile_pool(name="xs8", bufs=2) as xs8, \
             tc.tile_pool(name="work", bufs=2) as work, \
             tc.tile_pool(name="ps_t", bufs=2, space="PSUM") as ps_t, \
             tc.tile_pool(name="ps_v", bufs=2, space="PSUM") as ps_v, \
             tc.tile_pool(name="ps_y", bufs=3, space="PSUM") as ps_y:
            # ---- DMA schedule.  Ring rates are ~210 GB/s each (~0.21
            #      MB/us); SP's first packet lands ~1.5 us before ACT's.
            #      First bytes on each ring are the statistic-path operands
            #      (w18 quarters + x8 slab 0) so the PE can start at ~11 us;
            #      the X16 stream follows on SP; ACT carries the remaining
            #      x8 slabs and the small weights. ----
            w18_s = consts.tile([128, 8, 2, KV], F8, tag="w18")
            x16_t = [xs16.tile([128, 4, 16, SLAB], F16, tag="x16",
                               name=f"x16_{ss}") for ss in range(nss)]
            x8_t = [xs8.tile([128, 4, 8, 2, SLAB], F8, tag="x8",
                             name=f"x8_{ss}") for ss in range(nss)]
            # SP ring (consumption order: first w18 quarter, X16 stream)
            wf_s = consts.tile([128, 16, D], F16, tag="wf")
            mt_s = consts.tile([D, OUT], F16, tag="mt")
            nc.sync.dma_start(w18_s[:, 0:1], w18_d[:, 0:1])
            nc.sync.dma_start(w18_s[:, 1:2], w18_d[:, 1:2])
            for j in range(4):
                nc.sync.dma_start(x16_t[0][:, j], x16_d[0, :, j])
            for j in range(4):
                nc.sync.dma_start(x16_t[1][:, j], x16_d[1, :, j])
            # ACT ring (x8 slab 0 first, w18 quarters, weights/x8 in
            # PE-consumption order)
            nc.scalar.dma_start(x8_t[0][:, 0, 0:4], x8p_d[0, :, 0, 0:4])
            nc.scalar.dma_start(x8_t[0][:, 0, 4:8], x8p_d[0, :, 0, 4:8])
            for p in range(2, 8, 2):
                nc.scalar.dma_start(w18_s[:, p:p + 2], w18_d[:, p:p + 2])
            nc.scalar.dma_start(wf_s[:], wf_d)
            nc.scalar.dma_start(x8_t[0][:, 1], x8p_d[0, :, 1])
            nc.scalar.dma_start(x8_t[0][:, 2], x8p_d[0, :, 2])
            nc.scalar.dma_start(mt_s[:], mt_d)
            nc.scalar.dma_start(x8_t[0][:, 3], x8p_d[0, :, 3])
            for j in range(4):
                nc.scalar.dma_start(x8_t[1][:, j], x8p_d[1, :, j])
            if with_ba:
                bar8_s = consts.tile([1, KV], F8, tag="bar8")
                nc.scalar.dma_start(bar8_s[:], bar8_d)
                cvt_s = consts.tile([1, D], F16, tag="cvt")
                nc.scalar.dma_start(cvt_s[:], cvt_d)
                ones8_s = consts.tile([1, 128], F8, tag="ones8")
                nc.vector.memset(ones8_s[:], 1.0)
                ones16_s = consts.tile([1, SS_TOK], F16, tag="ones16")
                nc.vector.memset(ones16_s[:], 1.0)
            eps_s = consts.tile([128, 1], F32, tag="eps")
            nc.vector.memset(eps_s[:], EPS)

            # ---- PE warm-up / keep-alive: junk matmuls on an early-ready
            #      zero tile keep the HAM activity monitor from throttling
            #      the PE while data DMAs are in flight ----
            js = consts.tile([128, 512], F16, tag="js")
            nc.gpsimd.memset(js[:], 0.0)
            junkt = [ps_y.tile([128, 1024], F32, tag="py", bufs=2, name=f"junk{i}")
                     for i in range(2)]
            jn = [0]

            def junk(n):
                for _ in range(n):
                    i = jn[0] = jn[0] + 1
                    nc.tensor.matmul(junkt[i % 2][:, 0:512], js[:, 0:128],
                                     js[:], start=True, stop=True)

            junk(12)

            def norm_mm(ss, j, filler=False):
                # statistic matmul: T = X @ W1 in fp8 DoubleRow, token-major
                pt = ps_t.tile([128, KV], F32, tag="pt", name=f"pt{ss}_{j}")
                for p in range(8):
                    nc.tensor.matmul(pt[:], x8_t[ss][:, j, p], w18_s[:, p],
                                     start=(p == 0),
                                     stop=(p == 7 and not with_ba),
                                     perf_mode=DRMODE)
                    if filler and p % 2 == 1 and p < 7:
                        # first slab is paced by the w18 quarter DMAs; keep
                        # the PE clock warm between pair arrivals
                        junk(2)
                if with_ba:
                    # rank-1 row-broadcast of 64*kv_a_b into the accumulation
                    nc.tensor.matmul(pt[:], ones8_s[:], bar8_s[:],
                                     start=False, stop=True)
                return pt

            def stats(ss, j, pt):
                sq = work.tile([128, KV], F8, tag="sq", bufs=2)
                ssq = work.tile([128, 1], F32, tag="ssq", bufs=2)
                nc.scalar.activation(sq[:], pt[:], AF.Square,
                                     accum_out=ssq[:])
                rt = work.tile([128, 1], F32, tag="rt", bufs=2)
                nc.scalar.activation(rt[:], ssq[:], AF.Sqrt, bias=eps_s[:],
                                     scale=SQ_SCALE)
                sc = work.tile([128, 1], F32, tag="sc", bufs=8,
                               name=f"sc{ss}_{j}")
                nc.vector.reciprocal(sc[:], rt[:])
                return sc

            def value_mm(ss, h):
                # V.T = Wf.T @ X.T for one half-super-slab (2 slabs,
                # N=256), d-major; halves unblock as their X16 slabs land
                pv = ps_v.tile([128, 2 * SLAB], F32, tag="pv", bufs=2,
                               name=f"pv{ss}_{h}")
                for ck in range(16):
                    nc.tensor.matmul(pv[:], wf_s[:, ck],
                                     x16_t[ss][:, 2 * h:2 * h + 2, ck, :],
                                     start=(ck == 0),
                                     stop=(ck == 15 and not with_ba))
                if with_ba:
                    # rank-1: + (kv_a_b @ Wv') per-d constant over tokens
                    nc.tensor.matmul(pv[:], cvt_s[:],
                                     ones16_s[:, 0:2 * SLAB],
                                     start=False, stop=True)
                vts = work.tile([128, 2 * SLAB], F16, tag="vts", bufs=4,
                                name=f"vts{ss}_{h}")
                nc.scalar.activation(vts[:], pv[:], AF.Copy, bias=0.0,
                                     scale=1.0)
                return vts

            def step4(ss, j, vts, sc, last):
                # Y slab = s * (V.T-slab.T @ M.T); s applied during the
                # PSUM->SBUF copy (per-partition scalar on token-major out)
                t0 = (ss * 4 + j) * SLAB
                jh = (j % 2) * SLAB
                ysb = work.tile([128, OUT], F16, tag="ysb", bufs=4,
                                name=f"ysb{ss}_{j}")
                for h in range(2):
                    # two matmuls into one 2-bank PSUM tile, then a single
                    # wide scaled copy (DVE first half, ACT second half) so
                    # the copies never pace the matmuls
                    py = ps_y.tile([128, 1024], F32, tag="py", bufs=2,
                                   name=f"py{ss}_{j}_{h}")
                    for n in range(2):
                        nc.tensor.matmul(py[:, n * 512:(n + 1) * 512],
                                         vts[:, jh:jh + SLAB],
                                         mt_s[:, (2 * h + n) * 512:
                                              (2 * h + n + 1) * 512],
                                         start=True, stop=True)
                    ysl = ysb[:, h * 1024:(h + 1) * 1024]
                    if h == 0:
                        # DVE-produced half ships on the SP ring so its DMA
                        # trigger only waits on the DVE copy
                        nc.vector.tensor_scalar_mul(ysl, py[:], sc[:])
                        nc.sync.dma_start(y_d[t0:t0 + SLAB, 0:1024],
                                          ysb[:, 0:1024])
                    else:
                        nc.scalar.activation(ysl, py[:], AF.Copy, bias=0.0,
                                             scale=sc[:])
                        nc.scalar.dma_start(y_d[t0:t0 + SLAB, 1024:2048],
                                            ysb[:, 1024:2048])

            # ---- pipeline emission (PE program order == expected readiness
            #      order so the FIFO never head-of-line blocks; step4 calls
            #      are interleaved with norms so the PSUM->SBUF copies never
            #      pace the matmuls) ----
            scs = {}
            scs[(0, 0)] = stats(0, 0, norm_mm(0, 0, filler=True))
            junk(7)
            v0a = value_mm(0, 0)
            scs[(0, 1)] = stats(0, 1, norm_mm(0, 1))
            scs[(0, 2)] = stats(0, 2, norm_mm(0, 2))
            v0b = value_mm(0, 1)
            scs[(0, 3)] = stats(0, 3, norm_mm(0, 3))
            step4(0, 0, v0a, scs[(0, 0)], False)
            step4(0, 1, v0a, scs[(0, 1)], False)
            scs[(1, 0)] = stats(1, 0, norm_mm(1, 0))
            step4(0, 2, v0b, scs[(0, 2)], False)
            step4(0, 3, v0b, scs[(0, 3)], False)
            scs[(1, 1)] = stats(1, 1, norm_mm(1, 1))
            v1a = value_mm(1, 0)
            scs[(1, 2)] = stats(1, 2, norm_mm(1, 2))
            step4(1, 0, v1a, scs[(1, 0)], False)
            scs[(1, 3)] = stats(1, 3, norm_mm(1, 3))
            step4(1, 1, v1a, scs[(1, 1)], False)
            v1b = value_mm(1, 1)
            step4(1, 2, v1b, scs[(1, 2)], False)
            step4(1, 3, v1b, scs[(1, 3)], True)

    nc.compile()
    return nc


def _host_prep(inputs):
    """Fold weights, swizzle X into fp16 slab layout, shard across cores."""
    h = np.asarray(inputs["hidden_states"], dtype=np.float32)
    b, s, hid = h.shape
    assert hid == HID
    x = np.ascontiguousarray(h.reshape(b * s, hid))
    ntok = b * s
    tok = ntok // N_CORES
    nss = tok // SS_TOK

    kv_a_w = np.asarray(inputs["kv_a_w"], np.float32)
    kv_a_b = np.asarray(inputs["kv_a_b"], np.float32)
    kv_norm_w = np.asarray(inputs["kv_norm_w"], np.float32)
    kv_b_w = np.asarray(inputs["kv_b_w"], np.float32)
    kv_b_b = np.asarray(inputs["kv_b_b"], np.float32)
    o_w = np.asarray(inputs["o_w"], np.float32)

    W1 = np.ascontiguousarray(kv_a_w.T)                       # (2048, 512)
    Wvp = np.ascontiguousarray(
        (kv_b_w[D:2 * D] * (1.0 + kv_norm_w)[None, :]).T)     # (512, 128)
    Wf = W1 @ Wvp                                             # (2048, 128)
    Mh = o_w.reshape(HID, 16, D).sum(axis=1)                  # (2048, 128)

    w18 = np.clip(W1 * WSC, -240, 240).reshape(16, 128, KV) \
        .transpose(1, 0, 2).reshape(128, 8, 2, KV).astype(E4)
    wfh = np.ascontiguousarray(
        Wf.reshape(16, 128, D).transpose(1, 0, 2)).astype(np.float16)
    mth = np.ascontiguousarray(Mh.T).astype(np.float16)       # (128, 2048)

    with_ba = bool(np.any(kv_a_b != 0.0))
    consts = {"w18": w18, "wf": wfh, "mt": mth}
    if with_ba:
        consts["bar8"] = np.clip(kv_a_b * WSC, -240, 240) \
            .reshape(1, KV).astype(E4)
        consts["cvt"] = (kv_a_b @ Wvp).reshape(1, D).astype(np.float16)

    in_maps = []
    for i in range(N_CORES):
        shard = x[i * tok:(i + 1) * tok]
        x16 = np.ascontiguousarray(
            shard.reshape(nss, 4, SLAB, 16, 128).transpose(0, 4, 1, 3, 2)
        ).astype(np.float16)
        x8p = np.clip(x16.astype(np.float32), -240, 240) \
            .reshape(nss, 128, 4, 8, 2, SLAB).astype(E4)
        m = dict(consts)
        m["x16"] = x16
        m["x8p"] = x8p
        in_maps.append(m)

    bvrow = None
    if np.any(kv_b_b[D:2 * D] != 0.0):
        bvrow = (kv_b_b[D:2 * D] @ Mh.T).astype(np.float32)   # (2048,)

    def gather(results):
        y = np.concatenate([r["y"] for r in results], axis=0) \
            .astype(np.float32)
        if bvrow is not None:
            y += bvrow[None, :]
        return np.ascontiguousarray(y.reshape(b, s, HID))

    return in_maps, gather, with_ba, tok


def _run(inputs, trace=False, **spmd_kwargs):
    in_maps, gather, with_ba, tok = _host_prep(inputs)
    key = (tok, with_ba)
    if key not in _NC_CACHE:
        _NC_CACHE[key] = _build_nc(tok, with_ba)
    nc = _NC_CACHE[key]
    res = run_bass_kernel_spmd(nc, in_maps, core_ids=list(range(N_CORES)),
                               trace=trace, **spmd_kwargs)
    return gather(res.results), res


def kernel(**inputs) -> np.ndarray:
    y, _ = _run(inputs, trace=False)
    return y


# revision 35
# speedup vs baseline: 1.0174x; 1.0174x over previous
"""TRN2 Bass kernel for nn_Attention_5720896438407 (8-core data-parallel).

Mathematical collapse: the module computes SDPA over the *head* axis with a
single KV head (KV=1), so every attention weight is exactly 1.0 and the whole
module reduces to (see kernel_baseline.py.bak for the derivation)

    T  = hidden @ kv_a_w.T + kv_a_b                    # (ntok, 512)
    s  = rsqrt(mean(T^2, -1) + eps)                    # per-token RMS scale
    V  = (s*T) @ Wv' + bv,   Wv' = (kv_b_w[128:256] * (1+kv_norm_w)).T
    Y  = V @ M.T,            M   = o_w.reshape(2048, 16, 128).sum(1)

This version additionally *folds the value path past the RMS norm*: since s
is a per-token scalar,

    V = s * (hidden @ Wf) + bv,   Wf = kv_a_w.T @ Wv'   (2048 x 128)

so T is only needed for the statistic s.  That lets the dominant matmul
(hidden @ kv_a_w.T, 2048-dim contraction, 512 outputs) run in fp8 DoubleRow
mode (2 MACs/cell/cycle): fp8 quantization errors are independent across the
512 columns, so the *mean* of T^2 — and hence s — keeps ~0.2% accuracy while
the matmul runs at 2x rate.  The accurate value path is the cheap rank-128
fold (hidden @ Wf, fp16) plus the output projection (V @ M.T, fp16).
Numerically verified: ~2e-3 rel-fro error vs the fp32 reference.

Distribution: pure data-parallel over the 8192 tokens — 1024 per core, no
collectives.  Per core, tokens stream in 8 slabs of 128 (two 512-token
super-slabs); X ships twice (fp16 for the value path, pre-cast fp8 for the
statistic path — on-chip casting was measured far too slow on every
engine).  Per slab the PE does 8 DoubleRow MMs (statistic), 8 fp16 MMs
(value, batched per half-super-slab so it unblocks as X16 slabs land) and
4 fp16 MMs (output); ACT does the Square+accum statistic and half of each
Y PSUM->SBUF copy; DVE does s=1/sqrt and the other half, both applying the
per-token s during the copy.  The two HWDGE rings (SP/ACT) carry the
inputs in PE-consumption order and each Y half ships on the ring fed by
its producing engine.  Junk matmuls bridge the DMA-paced head so the HAM
activity monitor keeps the PE clock at 2.4 GHz (cold starts at 1.2 GHz and
re-throttles after ~3.4 us of idle — the dominant source of run-to-run
variance).
"""
import sys

sys.path.insert(0, "/opt/trn_rl_repo")

import numpy as np
import ml_dtypes
import concourse.tile as tile
from concourse import bacc, mybir
from concourse.bass_utils import run_bass_kernel_spmd

F32 = mybir.dt.float32
F16 = mybir.dt.float16
F8 = mybir.dt.float8e4
DRMODE = mybir.MatmulPerfMode.DoubleRow
AF = mybir.ActivationFunctionType

HID = 2048
KV = 512
D = 128
OUT = 2048
EPS = 1e-6
WSC = 64.0                    # fp8 scale on kv_a_w (entries ~0.02)
SQ_SCALE = 1.0 / (KV * WSC * WSC)
SLAB = 128                    # tokens per slab
SS_TOK = 512                  # tokens per super-slab (value-matmul batch)
N_CORES = 8
E4 = ml_dtypes.float8_e4m3

_NC_CACHE = {}


def _build_nc(tok, with_ba):
    nss = tok // SS_TOK
    assert tok % SS_TOK == 0 and nss == 2

    nc = bacc.Bacc("TRN2", target_bir_lowering=False, debug=False,
                   num_devices=1)

    x16_d = nc.dram_tensor("x16", (nss, 128, 4, 16, SLAB), F16,
                           kind="ExternalInput").ap()
    x8p_d = nc.dram_tensor("x8p", (nss, 128, 4, 8, 2, SLAB), F8,
                           kind="ExternalInput").ap()
    w18_d = nc.dram_tensor("w18", (128, 8, 2, KV), F8,
                           kind="ExternalInput").ap()
    wf_d = nc.dram_tensor("wf", (128, 16, D), F16, kind="ExternalInput").ap()
    mt_d = nc.dram_tensor("mt", (D, OUT), F16, kind="ExternalInput").ap()
    if with_ba:
        bar8_d = nc.dram_tensor("bar8", (1, KV), F8,
                                kind="ExternalInput").ap()
        cvt_d = nc.dram_tensor("cvt", (1, D), F16, kind="ExternalInput").ap()
    y_d = nc.dram_tensor("y", (tok, OUT), F16, kind="ExternalOutput").ap()

    with tile.TileContext(nc) as tc:
        with tc.tile_pool(name="consts", bufs=1) as consts, \
             tc.tile_pool(name="xs16", bufs=2) as xs16, \
             tc.tile_pool(name="xs8", bufs=2) as xs8, \
             tc.tile_pool(name="work", bufs=2) as work, \
             tc.tile_pool(name="ps_t", bufs=2, space="PSUM") as ps_t, \
             tc.tile_pool(name="ps_v", bufs=2, space="PSUM") as ps_v, \
             tc.tile_pool(name="ps_y", bufs=3, space="PSUM") as ps_y:
            # ---- DMA schedule.  Ring rates are ~210 GB/s each (~0.21
            #      MB/us); SP's first packet lands ~1.5 us before ACT's.
            #      First bytes on each ring are the statistic-path operands
            #      (w18 quarters + x8 slab 0) so the PE can start at ~11 us;
            #      the X16 stream follows on SP; ACT carries the remaining
            #      x8 slabs and the small weights. ----
            w18_s = consts.tile([128, 8, 2, KV], F8, tag="w18")
            x16_t = [xs16.tile([128, 4, 16, SLAB], F16, tag="x16",
                               name=f"x16_{ss}") for ss in range(nss)]
            x8_t = [xs8.tile([128, 4, 8, 2, SLAB], F8, tag="x8",
                             name=f"x8_{ss}") for ss in range(nss)]
            # SP ring (consumption order: first w18 quarter, X16 stream)
            wf_s = consts.tile([128, 16, D], F16, tag="wf")
            mt_s = consts.tile([D, OUT], F16, tag="mt")
            nc.sync.dma_start(w18_s[:, 0:2], w18_d[:, 0:2])
            for j in range(4):
                nc.sync.dma_start(x16_t[0][:, j], x16_d[0, :, j])
            for j in range(4):
                nc.sync.dma_start(x16_t[1][:, j], x16_d[1, :, j])
            # ACT ring (x8 slab 0 first, w18 quarters, weights/x8 in
            # PE-consumption order)
            nc.scalar.dma_start(x8_t[0][:, 0], x8p_d[0, :, 0])
            for p in range(2, 8, 2):
                nc.scalar.dma_start(w18_s[:, p:p + 2], w18_d[:, p:p + 2])
            nc.scalar.dma_start(wf_s[:], wf_d)
            nc.scalar.dma_start(x8_t[0][:, 1], x8p_d[0, :, 1])
            nc.scalar.dma_start(x8_t[0][:, 2], x8p_d[0, :, 2])
            nc.scalar.dma_start(mt_s[:], mt_d)
            nc.scalar.dma_start(x8_t[0][:, 3], x8p_d[0, :, 3])
            for j in range(4):
                nc.scalar.dma_start(x8_t[1][:, j], x8p_d[1, :, j])
            if with_ba:
                bar8_s = consts.tile([1, KV], F8, tag="bar8")
                nc.scalar.dma_start(bar8_s[:], bar8_d)
                cvt_s = consts.tile([1, D], F16, tag="cvt")
                nc.scalar.dma_start(cvt_s[:], cvt_d)
                ones8_s = consts.tile([1, 128], F8, tag="ones8")
                nc.vector.memset(ones8_s[:], 1.0)
                ones16_s = consts.tile([1, SS_TOK], F16, tag="ones16")
                nc.vector.memset(ones16_s[:], 1.0)
            eps_s = consts.tile([128, 1], F32, tag="eps")
            nc.vector.memset(eps_s[:], EPS)

            # ---- PE warm-up / keep-alive: junk matmuls on an early-ready
            #      zero tile keep the HAM activity monitor from throttling
            #      the PE while data DMAs are in flight ----
            js = consts.tile([128, 512], F16, tag="js")
            nc.gpsimd.memset(js[:], 0.0)
            junkt = [ps_y.tile([128, 1024], F32, tag="py", bufs=2, name=f"junk{i}")
                     for i in range(2)]
            jn = [0]

            def junk(n):
                for _ in range(n):
                    i = jn[0] = jn[0] + 1
                    nc.tensor.matmul(junkt[i % 2][:, 0:512], js[:, 0:128],
                                     js[:], start=True, stop=True)

            junk(12)

            def norm_mm(ss, j, filler=False):
                # statistic matmul: T = X @ W1 in fp8 DoubleRow, token-major
                pt = ps_t.tile([128, KV], F32, tag="pt", name=f"pt{ss}_{j}")
                for p in range(8):
                    nc.tensor.matmul(pt[:], x8_t[ss][:, j, p], w18_s[:, p],
                                     start=(p == 0),
                                     stop=(p == 7 and not with_ba),
                                     perf_mode=DRMODE)
                    if filler and p % 2 == 1 and p < 7:
                        # first slab is paced by the w18 quarter DMAs; keep
                        # the PE clock warm between pair arrivals
                        junk(2)
                if with_ba:
                    # rank-1 row-broadcast of 64*kv_a_b into the accumulation
                    nc.tensor.matmul(pt[:], ones8_s[:], bar8_s[:],
                                     start=False, stop=True)
                return pt

            def stats(ss, j, pt):
                sq = work.tile([128, KV], F8, tag="sq", bufs=2)
                ssq = work.tile([128, 1], F32, tag="ssq", bufs=2)
                nc.scalar.activation(sq[:], pt[:], AF.Square,
                                     accum_out=ssq[:])
                rt = work.tile([128, 1], F32, tag="rt", bufs=2)
                nc.scalar.activation(rt[:], ssq[:], AF.Sqrt, bias=eps_s[:],
                                     scale=SQ_SCALE)
                sc = work.tile([128, 1], F32, tag="sc", bufs=8,
                               name=f"sc{ss}_{j}")
                nc.vector.reciprocal(sc[:], rt[:])
                return sc

            def value_mm(ss, h):
                # V.T = Wf.T @ X.T for one half-super-slab (2 slabs,
                # N=256), d-major; halves unblock as their X16 slabs land
                pv = ps_v.tile([128, 2 * SLAB], F32, tag="pv", bufs=2,
                               name=f"pv{ss}_{h}")
                for ck in range(16):
                    nc.tensor.matmul(pv[:], wf_s[:, ck],
                                     x16_t[ss][:, 2 * h:2 * h + 2, ck, :],
                                     start=(ck == 0),
                                     stop=(ck == 15 and not with_ba))
                if with_ba:
                    # rank-1: + (kv_a_b @ Wv') per-d constant over tokens
                    nc.tensor.matmul(pv[:], cvt_s[:],
                                     ones16_s[:, 0:2 * SLAB],
                                     start=False, stop=True)
                vts = work.tile([128, 2 * SLAB], F16, tag="vts", bufs=4,
                                name=f"vts{ss}_{h}")
                nc.scalar.activation(vts[:], pv[:], AF.Copy, bias=0.0,
                                     scale=1.0)
                return vts

            def step4(ss, j, vts, sc, last):
                # Y slab = s * (V.T-slab.T @ M.T); s applied during the
                # PSUM->SBUF copy (per-partition scalar on token-major out)
                t0 = (ss * 4 + j) * SLAB
                jh = (j % 2) * SLAB
                ysb = work.tile([128, OUT], F16, tag="ysb", bufs=4,
                                name=f"ysb{ss}_{j}")
                for h in range(2):
                    # two matmuls into one 2-bank PSUM tile, then a single
                    # wide scaled copy (DVE first half, ACT second half) so
                    # the copies never pace the matmuls
                    py = ps_y.tile([128, 1024], F32, tag="py", bufs=2,
                                   name=f"py{ss}_{j}_{h}")
                    for n in range(2):
                        nc.tensor.matmul(py[:, n * 512:(n + 1) * 512],
                                         vts[:, jh:jh + SLAB],
                                         mt_s[:, (2 * h + n) * 512:
                                              (2 * h + n + 1) * 512],
                                         start=True, stop=True)
                    ysl = ysb[:, h * 1024:(h + 1) * 1024]
                    if h == 0:
                        # DVE-produced half ships on the SP ring so its DMA
                        # trigger only waits on the DVE copy
                        nc.vector.tensor_scalar_mul(ysl, py[:], sc[:])
                        nc.sync.dma_start(y_d[t0:t0 + SLAB, 0:1024],
                                          ysb[:, 0:1024])
                    else:
                        nc.scalar.activation(ysl, py[:], AF.Copy, bias=0.0,
                                             scale=sc[:])
                        nc.scalar.dma_start(y_d[t0:t0 + SLAB, 1024:2048],
                                            ysb[:, 1024:2048])

            # ---- pipeline emission (PE program order == expected readiness
            #      order so the FIFO never head-of-line blocks; step4 calls
            #      are interleaved with norms so the PSUM->SBUF copies never
            #      pace the matmuls) ----
            scs = {}
            scs[(0, 0)] = stats(0, 0, norm_mm(0, 0, filler=True))
            junk(7)
            v0a = value_mm(0, 0)
            scs[(0, 1)] = stats(0, 1, norm_mm(0, 1))
            scs[(0, 2)] = stats(0, 2, norm_mm(0, 2))
            v0b = value_mm(0, 1)
            scs[(0, 3)] = stats(0, 3, norm_mm(0, 3))
            step4(0, 0, v0a, scs[(0, 0)], False)
            step4(0, 1, v0a, scs[(0, 1)], False)
            scs[(1, 0)] = stats(1, 0, norm_mm(1, 0))
            step4(0, 2, v0b, scs[(0, 2)], False)
            step4(0, 3, v0b, scs[(0, 3)], False)
            scs[(1, 1)] = stats(1, 1, norm_mm(1, 1))
            v1a = value_mm(1, 0)
            scs[(1, 2)] = stats(1, 2, norm_mm(1, 2))
            step4(1, 0, v1a, scs[(1, 0)], False)
            scs[(1, 3)] = stats(1, 3, norm_mm(1, 3))
            step4(1, 1, v1a, scs[(1, 1)], False)
            v1b = value_mm(1, 1)
            step4(1, 2, v1b, scs[(1, 2)], False)
            step4(1, 3, v1b, scs[(1, 3)], True)

    nc.compile()
    return nc


def _host_prep(inputs):
    """Fold weights, swizzle X into fp16 slab layout, shard across cores."""
    h = np.asarray(inputs["hidden_states"], dtype=np.float32)
    b, s, hid = h.shape
    assert hid == HID
    x = np.ascontiguousarray(h.reshape(b * s, hid))
    ntok = b * s
    tok = ntok // N_CORES
    nss = tok // SS_TOK

    kv_a_w = np.asarray(inputs["kv_a_w"], np.float32)
    kv_a_b = np.asarray(inputs["kv_a_b"], np.float32)
    kv_norm_w = np.asarray(inputs["kv_norm_w"], np.float32)
    kv_b_w = np.asarray(inputs["kv_b_w"], np.float32)
    kv_b_b = np.asarray(inputs["kv_b_b"], np.float32)
    o_w = np.asarray(inputs["o_w"], np.float32)

    W1 = np.ascontiguousarray(kv_a_w.T)                       # (2048, 512)
    Wvp = np.ascontiguousarray(
        (kv_b_w[D:2 * D] * (1.0 + kv_norm_w)[None, :]).T)     # (512, 128)
    Wf = W1 @ Wvp                                             # (2048, 128)
    Mh = o_w.reshape(HID, 16, D).sum(axis=1)                  # (2048, 128)

    w18 = np.clip(W1 * WSC, -240, 240).reshape(16, 128, KV) \
        .transpose(1, 0, 2).reshape(128, 8, 2, KV).astype(E4)
    wfh = np.ascontiguousarray(
        Wf.reshape(16, 128, D).transpose(1, 0, 2)).astype(np.float16)
    mth = np.ascontiguousarray(Mh.T).astype(np.float16)       # (128, 2048)

    with_ba = bool(np.any(kv_a_b != 0.0))
    consts = {"w18": w18, "wf": wfh, "mt": mth}
    if with_ba:
        consts["bar8"] = np.clip(kv_a_b * WSC, -240, 240) \
            .reshape(1, KV).astype(E4)
        consts["cvt"] = (kv_a_b @ Wvp).reshape(1, D).astype(np.float16)

    in_maps = []
    for i in range(N_CORES):
        shard = x[i * tok:(i + 1) * tok]
        x16 = np.ascontiguousarray(
            shard.reshape(nss, 4, SLAB, 16, 128).transpose(0, 4, 1, 3, 2)
        ).astype(np.float16)
        x8p = np.clip(x16.astype(np.float32), -240, 240) \
            .reshape(nss, 128, 4, 8, 2, SLAB).astype(E4)
        m = dict(consts)
        m["x16"] = x16
        m["x8p"] = x8p
        in_maps.append(m)

    bvrow = None
    if np.any(kv_b_b[D:2 * D] != 0.0):
        bvrow = (kv_b_b[D:2 * D] @ Mh.T).astype(np.float32)   # (2048,)

    def gather(results):
        y = np.concatenate([r["y"] for r in results], axis=0) \
            .astype(np.float32)
        if bvrow is not None:
            y += bvrow[None, :]
        return np.ascontiguousarray(y.reshape(b, s, HID))

    return in_maps, gather, with_ba, tok


def _run(inputs, trace=False, **spmd_kwargs):
    in_maps, gather, with_ba, tok = _host_prep(inputs)
    key = (tok, with_ba)
    if key not in _NC_CACHE:
        _NC_CACHE[key] = _build_nc(tok, with_ba)
    nc = _NC_CACHE[key]
    res = run_bass_kernel_spmd(nc, in_maps, core_ids=list(range(N_CORES)),
                               trace=trace, **spmd_kwargs)
    return gather(res.results), res


def kernel(**inputs) -> np.ndarray:
    y, _ = _run(inputs, trace=False)
    return y
